# revision 28
# baseline (speedup 1.0000x reference)
"""Trainium2 Bass kernel for single-head attention (B=4, S=2048, D=H=1024).

Sharding: 8 cores = 4 batches x 2 sequence-halves. Each core projects
Q/K/V only for its OWN 1024-row half; the K^T and V halves are exchanged
with the pair partner via two pair-wise AllGather collectives (K first,
then V), each ~29us for 2 MiB and fully hidden behind the V/Q projection
matmuls. Scores/softmax/attnV/out then run on the core's own 1024 queries
against the full 2048-key sequence.

All matmuls single-pass fp16 (PSUM fp32 accumulation); rel l2 vs the fp32
reference is ~3e-3 (softmax logits ~N(0, 32^2), fp16 q/k rounding perturbs
logits by ~0.02 which the peaked softmax amplifies to ~2% absmax).

Per-core pipeline (v6):
  P0: x local half (2 MiB) + weights prefetched (ACT HWDGE queue for
      weights, SP for x; wv/wo/biases deferred past the first users).
  K:  KT_loc = Wk^T x -> staging -> kv_in_k -> AllGather -> KT full
      [h, 2048] SBUF tiles (both halves reloaded; position-independent
      so the SPMD program is identical across cores).
  V:  V_loc = x^T Wv -> staging -> kv_in_v -> AllGather -> V full
      [k, h] SBUF tiles. Runs while the K collective is in flight.
  Q:  Q = Wq^T x -> DRAM [H_T, QC, 128, 512] blocks. Runs while the V
      collective is in flight.
  B:  per 128-query tile: S = QT^T KT -> rowmax (negate) -> Exp(bias=-max,
      accum_out=den) -> En*recip -> 16 PE transposes (128x128, fp16 PSUM,
      packed 4 per bank) -> strided DVE copies into the per-chunk E^T
      supertile (no DRAM round trip; chunk-0 E^T complete mid-B).
  C:  per 512-query chunk: yT = V^T ET (+bv via softmax-sum=1),
      z^T = Wo^T yT (+bo) -> DRAM. Host transposes z^T back to [B,S,D].
"""

import sys

import numpy as np

for _p in ("/opt/trn_rl_repo",):
    if _p not in sys.path:
        sys.path.insert(0, _p)

import concourse.bass as bass
import concourse.masks as masks
import concourse.mybir as mybir
import concourse.tile as tile
from concourse.bass_utils import run_bass_kernel_spmd


def _install_profile_shims():
    """This image's `antenv` lacks `axon_hooks`, which run_bass_kernel_spmd
    imports for trace=True under axon; libaxon_pjrt.so has the NTFF symbols.
    Register a stand-in module wired to the ctypes hook, and neuter the
    artifact upload (zero-egress container)."""
    import types

    try:
        import antenv.axon_hooks  # noqa: F401
    except ImportError:
        hook = None
        try:
            import trn_agent_boot.trn_boot as _tb

            hook = _tb._ntff_profile_via_ctypes("/opt/axon/libaxon_pjrt.so")
        except Exception:
            hook = None
        import antenv

        m = types.ModuleType("antenv.axon_hooks")
        m.get_axon_ntff_profile_hook = lambda: hook
        m.set_axon_ntff_profile_hook = lambda h: None
        sys.modules["antenv.axon_hooks"] = m
        antenv.axon_hooks = m

    import concourse.bass_utils as _bu

    _bu.upload_artifacts = lambda tmpdir: tmpdir


_install_profile_shims()

B, S, D, H = 4, 2048, 1024, 1024
P = 128
NQ = 1024  # query rows per core == local key rows per core
D_T, H_T, S_T, Q_T = D // P, H // P, S // P, NQ // P
KC, QC, HC = S // 512, NQ // 512, H // 512
LC = NQ // 512  # local chunks

F32 = mybir.dt.float32
F16 = mybir.dt.float16
Ident = mybir.ActivationFunctionType.Identity
PAIRS = [[0, 1], [2, 3], [4, 5], [6, 7]]


def _split_multi_waits(nc, max_waits=1):
    """This container's walrus rejects >1 sync wait on NO_STRUCT opcodes
    (Drain/NoOp). Move extra waits onto dedicated single-wait NoOps inserted
    right before the offending instruction on the same engine."""
    for f in nc.m.functions:
        for bb in f.blocks:
            insts = bb.instructions
            i = 0
            while i < len(insts):
                ins = insts[i]
                si = ins.sync_info
                if si is not None and si.on_wait and len(si.on_wait) > max_waits:
                    waits = list(si.on_wait)
                    si.on_wait = waits[:max_waits]
                    ins.sync_info = si
                    for j, w in enumerate(waits[max_waits:]):
                        nop = mybir.InstNoOp(
                            name=f"{ins.name}-waitsplit-{j}",
                            engine=ins.engine,
                            bass_nofuse=True,
                            sync_info=mybir.SyncInfo(on_wait=[w], on_update=[]),
                        )
                        insts.insert(i, nop)
                        i += 1
                i += 1
            bb.instructions = insts


def _build(split_waits=True):
    nc = bass.Bass()

    def din(name, shape, dt=F16):
        return nc.declare_dram_parameter(name, shape, dt, isOutput=False)

    xT = din("xT", [D, NQ])  # this core's sequence half, [d, s_local]
    wq = din("wq", [D, H])
    wk = din("wk", [D, H])
    wv = din("wv", [D, H])
    wo = din("wo", [H, D])
    # biases pre-transposed on host to [128, n_tiles] so loads are contiguous
    bq, bk = din("bq", [P, H_T], F32), din("bk", [P, H_T], F32)
    bv, bo = din("bv", [P, H_T], F32), din("bo", [P, D_T], F32)
    zT = nc.declare_dram_parameter("zT", [D, NQ], F32, isOutput=True)

    with tile.TileContext(nc) as tc:
        with (
            tc.tile_pool(name="pers", bufs=1) as pers,
            tc.tile_pool(name="dram", bufs=1, space="DRAM") as dramp,
            tc.tile_pool(name="ps", bufs=6, space="PSUM") as psp,
            tc.tile_pool(name="pt", bufs=2, space="PSUM") as ptp,
        ):
            bias_q = pers.tile([P, H_T], F32, tag="bq", name="bq")
            bias_k = pers.tile([P, H_T], F32, tag="bk", name="bk")
            bias_v = pers.tile([P, H_T], F32, tag="bv", name="bv")
            bias_o = pers.tile([P, D_T], F32, tag="bo", name="bo")
            ident = pers.tile([P, P], F16, tag="ident", name="ident")
            masks.make_identity(nc, ident[:])

            # Q stays SBUF-resident: [h, q] tiles read directly as score
            # stationary operands in B.
            # Collective staging: local K^T/V halves out, both halves back.
            kv_in_k = dramp.tile([P, H_T * NQ], F16, tag="cink", name="cink")
            kv_in_v = dramp.tile([P, Q_T * H], F16, tag="cinv", name="cinv")
            kv_out_k = dramp.tile([2, P, H_T * NQ], F16, tag="coutk", name="coutk")
            kv_out_v = dramp.tile([2, P, Q_T * H], F16, tag="coutv", name="coutv")

            # Persistent SBUF: K^T, V, chunk-0 E^T supertile, weights.
            KT = [pers.tile([P, S], F16, tag=f"kt{t}", name=f"kt{t}") for t in range(H_T)]
            QS = [pers.tile([P, NQ], F16, tag=f"qs{t}", name=f"qs{t}") for t in range(H_T)]
            V = [pers.tile([P, H], F16, tag=f"v{s}", name=f"v{s}") for s in range(S_T)]
            ETa = pers.tile([P, S_T * 512], F16, tag="eta", name="eta")
            # wk per-d (streams into the first matmuls); wv/wo as one-DMA
            # supertiles (their consumers start late enough)
            wks = [pers.tile([P, H], F16, tag=f"wk{d}", name=f"wk{d}") for d in range(D_T)]
            wvs = pers.tile([P, D_T * H], F16, tag="wvs", name="wvs")
            wos = pers.tile([P, H_T * D], F16, tag="wos", name="wos")

            with tc.tile_pool(name="pqw", bufs=1) as pqw:
                wqs = pqw.tile([P, D_T * H], F16, tag="wqs", name="wqs")
                # biases first (tiny), then one supertile DMA per weight
                # matrix; x stream alone on the SP queue.
                nc.scalar.dma_start(out=bias_k[:], in_=bk[:, :])
                nc.scalar.dma_start(out=bias_q[:], in_=bq[:, :])
                nc.scalar.dma_start(out=bias_v[:], in_=bv[:, :])
                nc.scalar.dma_start(out=bias_o[:], in_=bo[:, :])
                for d in range(D_T):
                    nc.scalar.dma_start(out=wks[d][:], in_=wk[d * P : (d + 1) * P, :])
                nc.scalar.dma_start(
                    out=wqs[:].rearrange("p (d h) -> p d h", h=H),
                    in_=wq.rearrange("(d p) h -> p d h", p=P))
                nc.scalar.dma_start(
                    out=wvs[:].rearrange("p (d h) -> p d h", h=H),
                    in_=wv.rearrange("(d p) h -> p d h", p=P))
                nc.scalar.dma_start(
                    out=wos[:].rearrange("p (t d) -> p t d", d=D),
                    in_=wo.rearrange("(t p) d -> p t d", p=P))

                with tc.tile_pool(name="px", bufs=1) as px:
                    # x local half resident as 2 chunk supertiles [128, d*512]
                    # filled by per-d DMAs so the first matmuls can stream
                    xs = []
                    for c in range(LC):
                        cs = slice(c * 512, (c + 1) * 512)
                        t_ = px.tile([P, D_T * 512], F16, tag=f"x{c}",
                                     name=f"x{c}")
                        for d in range(D_T):
                            nc.sync.dma_start(
                                out=t_[:, d * 512 : (d + 1) * 512],
                                in_=xT[d * P : (d + 1) * P, cs])
                        xs.append(t_)

                    # ---- K: local K^T -> staging -> AllGather ------------
                    KTW = px.tile([P, H_T * NQ], F16, tag="stg", name="ktw")
                    for kc in range(LC):
                        for t in range(H_T):
                            ps = psp.tile([P, 512], F32, tag="ps", name="ps")
                            for d in range(D_T):
                                nc.tensor.matmul(
                                    ps[:],
                                    wks[d][:, t * P : (t + 1) * P],
                                    xs[kc][:, d * 512 : (d + 1) * 512],
                                    start=(d == 0), stop=(d == D_T - 1))
                            ws = slice(t * NQ + kc * 512, t * NQ + (kc + 1) * 512)
                            nc.scalar.activation(KTW[:, ws], ps[:], Ident,
                                                 bias=bias_k[:, t : t + 1])
                    nc.gpsimd.dma_start(out=kv_in_k[:, :], in_=KTW[:])
                    nc.gpsimd.collective_compute(
                        "AllGather", mybir.AluOpType.bypass,
                        replica_groups=PAIRS,
                        ins=[kv_in_k[:, :]],
                        outs=[kv_out_k[:, :, :]],
                    )
                    for t in range(H_T):
                        for p_ in range(2):
                            nc.sync.dma_start(
                                out=KT[t][:, p_ * NQ : (p_ + 1) * NQ],
                                in_=kv_out_k[p_, :, t * NQ : (t + 1) * NQ])

                    # ---- Q: local queries -> QS (SBUF resident) ----------
                    for qc in range(QC):
                        qcs = slice(qc * 512, (qc + 1) * 512)
                        for t in range(H_T):
                            ps = psp.tile([P, 512], F32, tag="ps", name="ps")
                            for d in range(D_T):
                                nc.tensor.matmul(
                                    ps[:],
                                    wqs[:, d * H + t * P : d * H + (t + 1) * P],
                                    xs[qc][:, d * 512 : (d + 1) * 512],
                                    start=(d == 0), stop=(d == D_T - 1))
                            nc.scalar.activation(QS[t][:, qcs], ps[:], Ident,
                                                 bias=bias_q[:, t : t + 1])

                    # ---- V: local V -> staging -> AllGather --------------
                    VTW = px.tile([P, Q_T * H], F16, tag="stg", name="vtw")
                    for kc in range(LC):
                        for si in range(4):
                            sl = kc * 4 + si
                            ksl = slice(si * P, (si + 1) * P)
                            for hc in range(HC):
                                hcs = slice(hc * 512, (hc + 1) * 512)
                                ps = psp.tile([P, 512], F32, tag="ps", name="ps")
                                for d in range(D_T):
                                    nc.tensor.matmul(
                                        ps[:],
                                        xs[kc][:, d * 512 + si * P : d * 512 + (si + 1) * P],
                                        wvs[:, d * H + hc * 512 : d * H + (hc + 1) * 512],
                                        start=(d == 0), stop=(d == D_T - 1))
                                # no +bv here: y = A(V + 1 bv^T) = AV + bv
                                # since softmax rows sum to 1; added in C.
                                ws = slice(sl * H + hc * 512,
                                           sl * H + (hc + 1) * 512)
                                nc.vector.tensor_copy(VTW[:, ws], ps[:])
                    nc.gpsimd.dma_start(out=kv_in_v[:, :], in_=VTW[:])
                    nc.gpsimd.collective_compute(
                        "AllGather", mybir.AluOpType.bypass,
                        replica_groups=PAIRS,
                        ins=[kv_in_v[:, :]],
                        outs=[kv_out_v[:, :, :]],
                    )
                    for s in range(S_T):
                        p_, sl = divmod(s, Q_T)
                        nc.sync.dma_start(
                            out=V[s][:],
                            in_=kv_out_v[p_, :, sl * H : (sl + 1) * H])


            # ---- B: scores + softmax + on-chip transpose -----------------
            with tc.tile_pool(name="pe2", bufs=1) as pe2:
                ETb = pe2.tile([P, S_T * 512], F16, tag="etb", name="etb")
                with tc.tile_pool(name="pb", bufs=1) as pb:
                    # qt3 last: chunk-1's E^T (qt4-7) completes one iteration
                    # early, so C's chunk-1 attnV hides qt3's softmax chain.
                    # Transposes are emitted one iteration BEHIND their qt:
                    # they run on the tensor engine, and emitting them right
                    # after their own qt's scores would head-of-line-block
                    # the next qt's scores on the softmax-chain latency.
                    def transpose_emit(qt, Enn):
                        ET = ETa if qt < 4 else ETb
                        j = qt % 4
                        for a in range(4):
                            pst = ptp.tile([P, 512], F16, tag="pst", name="pst")
                            for i in range(4):
                                s = 4 * a + i
                                nc.tensor.transpose(
                                    pst[:, i * P : (i + 1) * P],
                                    Enn[:, s * P : (s + 1) * P],
                                    ident[:])
                            src = pst[:].rearrange("p (i c) -> p i c", c=P)
                            dst = ET[:].rearrange("p (s c) -> p s c", c=512)[
                                :, 4 * a : 4 * a + 4, j * P : (j + 1) * P]
                            nc.vector.tensor_copy(dst, src)

                    prev = None
                    for qt in (0, 1, 2, 4, 5, 6, 7, 3):
                        qs_ = slice(qt * P, (qt + 1) * P)
                        Ssb = pb.tile([P, S], F32, tag="Ssb", name="Ssb", bufs=2)
                        for kc in range(KC):
                            cs = slice(kc * 512, (kc + 1) * 512)
                            ps = psp.tile([P, 512], F32, tag="ps", name="ps")
                            for t in range(H_T):
                                nc.tensor.matmul(
                                    ps[:], QS[t][:, qs_], KT[t][:, cs],
                                    start=(t == 0), stop=(t == H_T - 1))
                            nc.vector.tensor_copy(Ssb[:, cs], ps[:])
                        if prev is not None:
                            transpose_emit(*prev)
                        nmx = pb.tile([P, 1], F32, tag="nmx", name="nmx", bufs=2)
                        nc.vector.reduce_max(nmx[:], Ssb[:],
                                             axis=mybir.AxisListType.X,
                                             negate=True)
                        En = pb.tile([P, S], F16, tag="En", name="En")
                        den = pb.tile([P, 1], F32, tag="den", name="den", bufs=2)
                        nc.scalar.activation(
                            En[:], Ssb[:], mybir.ActivationFunctionType.Exp,
                            bias=nmx[:], accum_out=den[:])
                        rec = pb.tile([P, 1], F32, tag="rec", name="rec", bufs=2)
                        nc.vector.reciprocal(rec[:], den[:])
                        Enn = pe2.tile([P, S], F16, tag="Enn", name="Enn", bufs=2)
                        nc.scalar.mul(Enn[:], En[:], rec[:])
                        prev = (qt, Enn)

                # ---- C: per q-chunk: yT = V^T ET, z = Wo^T yT ------------
                with tc.tile_pool(name="pc", bufs=1) as pc:
                    for ci, qc in enumerate((1, 0)):
                        if ci == 1:
                            transpose_emit(*prev)  # qt3, hidden under chunk 1
                        cs = slice(qc * 512, (qc + 1) * 512)
                        ET = ETa if qc == 0 else ETb
                        ycs = []
                        for t in range(H_T):
                            hs = slice(t * P, (t + 1) * P)
                            ps = psp.tile([P, 512], F32, tag="ps", name="ps")
                            for s in range(S_T):
                                nc.tensor.matmul(
                                    ps[:], V[s][:, hs],
                                    ET[:, s * 512 : (s + 1) * 512],
                                    start=(s == 0), stop=(s == S_T - 1))
                            yc = pc.tile([P, 512], F16, tag=f"yc{t}",
                                         name=f"yc{t}", bufs=2)
                            nc.scalar.activation(yc[:], ps[:], Ident,
                                                 bias=bias_v[:, t : t + 1])
                            ycs.append(yc)
                        for td in range(D_T):
                            ds_ = slice(td * P, (td + 1) * P)
                            ps = psp.tile([P, 512], F32, tag="ps", name="ps")
                            for t in range(H_T):
                                nc.tensor.matmul(
                                    ps[:],
                                    wos[:, t * D + td * P : t * D + (td + 1) * P],
                                    ycs[t][:],
                                    start=(t == 0), stop=(t == H_T - 1))
                            zsb = pc.tile([P, 512], F32, tag="zsb", name="zsb",
                                          bufs=2)
                            nc.scalar.activation(zsb[:], ps[:], Ident,
                                                 bias=bias_o[:, td : td + 1])
                            nc.sync.dma_start(out=zT[ds_, cs], in_=zsb[:])

    if split_waits:
        _split_multi_waits(nc)
    return nc


_NC = {}


def _get_nc():
    if "v6" not in _NC:
        _NC["v6"] = _build()
    return _NC["v6"]


def _in_maps(x, Wq, bq, Wk, bk, Wv, bv, Wo, bo):
    x = np.asarray(x, np.float32)
    xT = np.transpose(x, (0, 2, 1)).astype(np.float16)  # [B, D, S]
    com = {
        "wq": np.asarray(Wq, np.float16),
        "wk": np.asarray(Wk, np.float16),
        "wv": np.asarray(Wv, np.float16),
        "wo": np.asarray(Wo, np.float16),
        "bq": np.ascontiguousarray(np.asarray(bq, np.float32).reshape(H_T, P).T),
        "bk": np.ascontiguousarray(np.asarray(bk, np.float32).reshape(H_T, P).T),
        "bv": np.ascontiguousarray(np.asarray(bv, np.float32).reshape(H_T, P).T),
        "bo": np.ascontiguousarray(np.asarray(bo, np.float32).reshape(D_T, P).T),
    }
    maps = []
    for c in range(8):
        b, h = divmod(c, 2)
        m = dict(com)
        m["xT"] = np.ascontiguousarray(xT[b][:, h * NQ : (h + 1) * NQ])
        maps.append(m)
    return maps


def kernel(x, Wq, bq, Wk, bk, Wv, bv, Wo, bo, _trace=False, _precise=None):
    nc = _get_nc()
    maps = _in_maps(x, Wq, bq, Wk, bk, Wv, bv, Wo, bo)
    res = run_bass_kernel_spmd(nc, maps, list(range(8)), trace=_trace)
    out = np.empty((B, S, D), np.float32)
    for c in range(8):
        b, h = divmod(c, 2)
        out[b, h * NQ : (h + 1) * NQ, :] = res.results[c]["zT"].T
    if _trace:
        kernel.last_exec_time_ns = res.exec_time_ns
        kernel.last_profile = res
    return out


# revision 29
# speedup vs baseline: 1.0225x; 1.0225x over previous
"""Trainium2 Bass kernel for single-head attention (B=4, S=2048, D=H=1024).

Sharding: 8 cores = 4 batches x 2 sequence-halves. Each core projects
Q/K/V only for its OWN 1024-row half; the K^T and V halves are exchanged
with the pair partner via two pair-wise AllGather collectives (K first,
then V), each ~29us for 2 MiB and fully hidden behind the V/Q projection
matmuls. Scores/softmax/attnV/out then run on the core's own 1024 queries
against the full 2048-key sequence.

All matmuls single-pass fp16 (PSUM fp32 accumulation); rel l2 vs the fp32
reference is ~3e-3 (softmax logits ~N(0, 32^2), fp16 q/k rounding perturbs
logits by ~0.02 which the peaked softmax amplifies to ~2% absmax).

Per-core pipeline (v6):
  P0: x local half (2 MiB) + weights prefetched (ACT HWDGE queue for
      weights, SP for x; wv/wo/biases deferred past the first users).
  K:  KT_loc = Wk^T x -> staging -> kv_in_k -> AllGather -> KT full
      [h, 2048] SBUF tiles (both halves reloaded; position-independent
      so the SPMD program is identical across cores).
  V:  V_loc = x^T Wv -> staging -> kv_in_v -> AllGather -> V full
      [k, h] SBUF tiles. Runs while the K collective is in flight.
  Q:  Q = Wq^T x -> DRAM [H_T, QC, 128, 512] blocks. Runs while the V
      collective is in flight.
  B:  per 128-query tile: S = QT^T KT -> rowmax (negate) -> Exp(bias=-max,
      accum_out=den) -> En*recip -> 16 PE transposes (128x128, fp16 PSUM,
      packed 4 per bank) -> strided DVE copies into the per-chunk E^T
      supertile (no DRAM round trip; chunk-0 E^T complete mid-B).
  C:  per 512-query chunk: yT = V^T ET (+bv via softmax-sum=1),
      z^T = Wo^T yT (+bo) -> DRAM. Host transposes z^T back to [B,S,D].
"""

import sys

import numpy as np

for _p in ("/opt/trn_rl_repo",):
    if _p not in sys.path:
        sys.path.insert(0, _p)

import concourse.bass as bass
import concourse.masks as masks
import concourse.mybir as mybir
import concourse.tile as tile
from concourse.bass_utils import run_bass_kernel_spmd


def _install_profile_shims():
    """This image's `antenv` lacks `axon_hooks`, which run_bass_kernel_spmd
    imports for trace=True under axon; libaxon_pjrt.so has the NTFF symbols.
    Register a stand-in module wired to the ctypes hook, and neuter the
    artifact upload (zero-egress container)."""
    import types

    try:
        import antenv.axon_hooks  # noqa: F401
    except ImportError:
        hook = None
        try:
            import trn_agent_boot.trn_boot as _tb

            hook = _tb._ntff_profile_via_ctypes("/opt/axon/libaxon_pjrt.so")
        except Exception:
            hook = None
        import antenv

        m = types.ModuleType("antenv.axon_hooks")
        m.get_axon_ntff_profile_hook = lambda: hook
        m.set_axon_ntff_profile_hook = lambda h: None
        sys.modules["antenv.axon_hooks"] = m
        antenv.axon_hooks = m

    import concourse.bass_utils as _bu

    _bu.upload_artifacts = lambda tmpdir: tmpdir


_install_profile_shims()

B, S, D, H = 4, 2048, 1024, 1024
P = 128
NQ = 1024  # query rows per core == local key rows per core
D_T, H_T, S_T, Q_T = D // P, H // P, S // P, NQ // P
KC, QC, HC = S // 512, NQ // 512, H // 512
LC = NQ // 512  # local chunks

F32 = mybir.dt.float32
F16 = mybir.dt.float16
Ident = mybir.ActivationFunctionType.Identity
PAIRS = [[0, 1], [2, 3], [4, 5], [6, 7]]


def _split_multi_waits(nc, max_waits=1):
    """This container's walrus rejects >1 sync wait on NO_STRUCT opcodes
    (Drain/NoOp). Move extra waits onto dedicated single-wait NoOps inserted
    right before the offending instruction on the same engine."""
    for f in nc.m.functions:
        for bb in f.blocks:
            insts = bb.instructions
            i = 0
            while i < len(insts):
                ins = insts[i]
                si = ins.sync_info
                if si is not None and si.on_wait and len(si.on_wait) > max_waits:
                    waits = list(si.on_wait)
                    si.on_wait = waits[:max_waits]
                    ins.sync_info = si
                    for j, w in enumerate(waits[max_waits:]):
                        nop = mybir.InstNoOp(
                            name=f"{ins.name}-waitsplit-{j}",
                            engine=ins.engine,
                            bass_nofuse=True,
                            sync_info=mybir.SyncInfo(on_wait=[w], on_update=[]),
                        )
                        insts.insert(i, nop)
                        i += 1
                i += 1
            bb.instructions = insts


def _build(split_waits=True):
    nc = bass.Bass()

    def din(name, shape, dt=F16):
        return nc.declare_dram_parameter(name, shape, dt, isOutput=False)

    xT = din("xT", [D, NQ])  # this core's sequence half, [d, s_local]
    wq = din("wq", [D, H])
    wk = din("wk", [D, H])
    wv = din("wv", [D, H])
    wo = din("wo", [H, D])
    # biases pre-transposed on host to [128, n_tiles] so loads are contiguous
    bq, bk = din("bq", [P, H_T], F32), din("bk", [P, H_T], F32)
    bv, bo = din("bv", [P, H_T], F32), din("bo", [P, D_T], F32)
    zT = nc.declare_dram_parameter("zT", [D, NQ], F32, isOutput=True)

    with tile.TileContext(nc) as tc:
        with (
            tc.tile_pool(name="pers", bufs=1) as pers,
            tc.tile_pool(name="dram", bufs=1, space="DRAM") as dramp,
            tc.tile_pool(name="ps", bufs=6, space="PSUM") as psp,
            tc.tile_pool(name="pt", bufs=2, space="PSUM") as ptp,
        ):
            bias_q = pers.tile([P, H_T], F32, tag="bq", name="bq")
            bias_k = pers.tile([P, H_T], F32, tag="bk", name="bk")
            bias_v = pers.tile([P, H_T], F32, tag="bv", name="bv")
            bias_o = pers.tile([P, D_T], F32, tag="bo", name="bo")
            ident = pers.tile([P, P], F16, tag="ident", name="ident")
            masks.make_identity(nc, ident[:])

            # Q stays SBUF-resident: [h, q] tiles read directly as score
            # stationary operands in B.
            # Collective staging: local K^T/V halves out, both halves back.
            kv_in_k = dramp.tile([P, H_T * NQ], F16, tag="cink", name="cink")
            kv_in_v = dramp.tile([P, Q_T * H], F16, tag="cinv", name="cinv")
            kv_out_k = dramp.tile([2, P, H_T * NQ], F16, tag="coutk", name="coutk")
            kv_out_v = dramp.tile([2, P, Q_T * H], F16, tag="coutv", name="coutv")

            # Persistent SBUF: K^T, V, chunk-0 E^T supertile, weights.
            KT = [pers.tile([P, S], F16, tag=f"kt{t}", name=f"kt{t}") for t in range(H_T)]
            QS = [pers.tile([P, NQ], F16, tag=f"qs{t}", name=f"qs{t}") for t in range(H_T)]
            V = [pers.tile([P, H], F16, tag=f"v{s}", name=f"v{s}") for s in range(S_T)]
            ETa = pers.tile([P, S_T * 512], F16, tag="eta", name="eta")
            # wk per-d (streams into the first matmuls); wv/wo as one-DMA
            # supertiles (their consumers start late enough)
            wks = [pers.tile([P, H], F16, tag=f"wk{d}", name=f"wk{d}") for d in range(D_T)]
            wvs = pers.tile([P, D_T * H], F16, tag="wvs", name="wvs")
            wos = pers.tile([P, H_T * D], F16, tag="wos", name="wos")

            with tc.tile_pool(name="pqw", bufs=1) as pqw:
                wqs = pqw.tile([P, D_T * H], F16, tag="wqs", name="wqs")
                # biases first (tiny), then one supertile DMA per weight
                # matrix; x stream alone on the SP queue.
                nc.scalar.dma_start(out=bias_k[:], in_=bk[:, :])
                nc.scalar.dma_start(out=bias_q[:], in_=bq[:, :])
                nc.scalar.dma_start(out=bias_v[:], in_=bv[:, :])
                nc.scalar.dma_start(out=bias_o[:], in_=bo[:, :])
                for d in range(D_T):
                    nc.scalar.dma_start(out=wks[d][:], in_=wk[d * P : (d + 1) * P, :])
                nc.scalar.dma_start(
                    out=wqs[:].rearrange("p (d h) -> p d h", h=H),
                    in_=wq.rearrange("(d p) h -> p d h", p=P))
                nc.scalar.dma_start(
                    out=wvs[:].rearrange("p (d h) -> p d h", h=H),
                    in_=wv.rearrange("(d p) h -> p d h", p=P))
                nc.scalar.dma_start(
                    out=wos[:].rearrange("p (t d) -> p t d", d=D),
                    in_=wo.rearrange("(t p) d -> p t d", p=P))

                with tc.tile_pool(name="px", bufs=1) as px:
                    # x local half resident as 2 chunk supertiles [128, d*512]
                    # filled by per-d DMAs so the first matmuls can stream
                    xs = []
                    for c in range(LC):
                        cs = slice(c * 512, (c + 1) * 512)
                        t_ = px.tile([P, D_T * 512], F16, tag=f"x{c}",
                                     name=f"x{c}")
                        for d in range(D_T):
                            nc.sync.dma_start(
                                out=t_[:, d * 512 : (d + 1) * 512],
                                in_=xT[d * P : (d + 1) * P, cs])
                        xs.append(t_)

                    # ---- K: local K^T -> staging -> AllGather ------------
                    KTW = px.tile([P, H_T * NQ], F16, tag="stg", name="ktw")
                    for kc in range(LC):
                        for t in range(H_T):
                            ps = psp.tile([P, 512], F32, tag="ps", name="ps")
                            for d in range(D_T):
                                nc.tensor.matmul(
                                    ps[:],
                                    wks[d][:, t * P : (t + 1) * P],
                                    xs[kc][:, d * 512 : (d + 1) * 512],
                                    start=(d == 0), stop=(d == D_T - 1))
                            ws = slice(t * NQ + kc * 512, t * NQ + (kc + 1) * 512)
                            nc.scalar.activation(KTW[:, ws], ps[:], Ident,
                                                 bias=bias_k[:, t : t + 1])
                    nc.gpsimd.dma_start(out=kv_in_k[:, :], in_=KTW[:])
                    nc.gpsimd.collective_compute(
                        "AllGather", mybir.AluOpType.bypass,
                        replica_groups=PAIRS,
                        ins=[kv_in_k[:, :]],
                        outs=[kv_out_k[:, :, :]],
                    )
                    for t in range(H_T):
                        for p_ in range(2):
                            nc.sync.dma_start(
                                out=KT[t][:, p_ * NQ : (p_ + 1) * NQ],
                                in_=kv_out_k[p_, :, t * NQ : (t + 1) * NQ])

                    # ---- Q: local queries -> QS (SBUF resident) ----------
                    for qc in range(QC):
                        qcs = slice(qc * 512, (qc + 1) * 512)
                        for t in range(H_T):
                            ps = psp.tile([P, 512], F32, tag="ps", name="ps")
                            for d in range(D_T):
                                nc.tensor.matmul(
                                    ps[:],
                                    wqs[:, d * H + t * P : d * H + (t + 1) * P],
                                    xs[qc][:, d * 512 : (d + 1) * 512],
                                    start=(d == 0), stop=(d == D_T - 1))
                            nc.scalar.activation(QS[t][:, qcs], ps[:], Ident,
                                                 bias=bias_q[:, t : t + 1])

                    # ---- V: local V -> staging -> AllGather --------------
                    VTW = px.tile([P, Q_T * H], F16, tag="stg", name="vtw")
                    for kc in range(LC):
                        for si in range(4):
                            sl = kc * 4 + si
                            ksl = slice(si * P, (si + 1) * P)
                            for hc in range(HC):
                                hcs = slice(hc * 512, (hc + 1) * 512)
                                ps = psp.tile([P, 512], F32, tag="ps", name="ps")
                                for d in range(D_T):
                                    nc.tensor.matmul(
                                        ps[:],
                                        xs[kc][:, d * 512 + si * P : d * 512 + (si + 1) * P],
                                        wvs[:, d * H + hc * 512 : d * H + (hc + 1) * 512],
                                        start=(d == 0), stop=(d == D_T - 1))
                                # no +bv here: y = A(V + 1 bv^T) = AV + bv
                                # since softmax rows sum to 1; added in C.
                                ws = slice(sl * H + hc * 512,
                                           sl * H + (hc + 1) * 512)
                                nc.vector.tensor_copy(VTW[:, ws], ps[:])
                    nc.gpsimd.dma_start(out=kv_in_v[:, :], in_=VTW[:])
                    nc.gpsimd.collective_compute(
                        "AllGather", mybir.AluOpType.bypass,
                        replica_groups=PAIRS,
                        ins=[kv_in_v[:, :]],
                        outs=[kv_out_v[:, :, :]],
                    )
                    for s in range(S_T):
                        p_, sl = divmod(s, Q_T)
                        nc.sync.dma_start(
                            out=V[s][:],
                            in_=kv_out_v[p_, :, sl * H : (sl + 1) * H])


            # ---- B: scores + softmax + on-chip transpose -----------------
            with tc.tile_pool(name="pe2", bufs=1) as pe2:
                ETb = pe2.tile([P, S_T * 512], F16, tag="etb", name="etb")
                with tc.tile_pool(name="pb", bufs=1) as pb:
                    for qt in range(Q_T):
                        qs_ = slice(qt * P, (qt + 1) * P)
                        Ssb = pb.tile([P, S], F32, tag="Ssb", name="Ssb", bufs=2)
                        for kc in range(KC):
                            cs = slice(kc * 512, (kc + 1) * 512)
                            ps = psp.tile([P, 512], F32, tag="ps", name="ps")
                            for t in range(H_T):
                                nc.tensor.matmul(
                                    ps[:], QS[t][:, qs_], KT[t][:, cs],
                                    start=(t == 0), stop=(t == H_T - 1))
                            nc.vector.tensor_copy(Ssb[:, cs], ps[:])
                        nmx = pb.tile([P, 1], F32, tag="nmx", name="nmx", bufs=2)
                        nc.vector.reduce_max(nmx[:], Ssb[:],
                                             axis=mybir.AxisListType.X,
                                             negate=True)
                        En = pb.tile([P, S], F16, tag="En", name="En")
                        den = pb.tile([P, 1], F32, tag="den", name="den", bufs=2)
                        nc.scalar.activation(
                            En[:], Ssb[:], mybir.ActivationFunctionType.Exp,
                            bias=nmx[:], accum_out=den[:])
                        rec = pb.tile([P, 1], F32, tag="rec", name="rec", bufs=2)
                        nc.vector.reciprocal(rec[:], den[:])
                        Enn = pb.tile([P, S], F16, tag="Enn", name="Enn", bufs=2)
                        nc.scalar.mul(Enn[:], En[:], rec[:])
                        # On-chip transpose: Enn [q, k] -> ET [k, q], 16
                        # 128x128 PE transposes packed 4 per fp16 PSUM bank,
                        # then one strided DVE copy per pack of 4.
                        ET = ETa if qt < 4 else ETb
                        j = qt % 4
                        for a in range(4):
                            pst = ptp.tile([P, 512], F16, tag="pst", name="pst")
                            for i in range(4):
                                s = 4 * a + i
                                nc.tensor.transpose(
                                    pst[:, i * P : (i + 1) * P],
                                    Enn[:, s * P : (s + 1) * P],
                                    ident[:])
                            src = pst[:].rearrange("p (i c) -> p i c", c=P)
                            dst = ET[:].rearrange("p (s c) -> p s c", c=512)[
                                :, 4 * a : 4 * a + 4, j * P : (j + 1) * P]
                            nc.vector.tensor_copy(dst, src)

                # ---- C: per q-chunk: yT = V^T ET, z = Wo^T yT ------------
                with tc.tile_pool(name="pc", bufs=1) as pc:
                    for qc in range(QC):
                        cs = slice(qc * 512, (qc + 1) * 512)
                        ET = ETa if qc == 0 else ETb
                        ycs = []
                        for t in range(H_T):
                            hs = slice(t * P, (t + 1) * P)
                            ps = psp.tile([P, 512], F32, tag="ps", name="ps")
                            for s in range(S_T):
                                nc.tensor.matmul(
                                    ps[:], V[s][:, hs],
                                    ET[:, s * 512 : (s + 1) * 512],
                                    start=(s == 0), stop=(s == S_T - 1))
                            yc = pc.tile([P, 512], F16, tag=f"yc{t}",
                                         name=f"yc{t}", bufs=2)
                            nc.scalar.activation(yc[:], ps[:], Ident,
                                                 bias=bias_v[:, t : t + 1])
                            ycs.append(yc)
                        for td in range(D_T):
                            ds_ = slice(td * P, (td + 1) * P)
                            ps = psp.tile([P, 512], F32, tag="ps", name="ps")
                            for t in range(H_T):
                                nc.tensor.matmul(
                                    ps[:],
                                    wos[:, t * D + td * P : t * D + (td + 1) * P],
                                    ycs[t][:],
                                    start=(t == 0), stop=(t == H_T - 1))
                            zsb = pc.tile([P, 512], F32, tag="zsb", name="zsb",
                                          bufs=2)
                            nc.scalar.activation(zsb[:], ps[:], Ident,
                                                 bias=bias_o[:, td : td + 1])
                            nc.sync.dma_start(out=zT[ds_, cs], in_=zsb[:])

    if split_waits:
        _split_multi_waits(nc)
    return nc


_NC = {}


def _get_nc():
    if "v6" not in _NC:
        _NC["v6"] = _build()
    return _NC["v6"]


def _in_maps(x, Wq, bq, Wk, bk, Wv, bv, Wo, bo):
    x = np.asarray(x, np.float32)
    xT = np.transpose(x, (0, 2, 1)).astype(np.float16)  # [B, D, S]
    com = {
        "wq": np.asarray(Wq, np.float16),
        "wk": np.asarray(Wk, np.float16),
        "wv": np.asarray(Wv, np.float16),
        "wo": np.asarray(Wo, np.float16),
        "bq": np.ascontiguousarray(np.asarray(bq, np.float32).reshape(H_T, P).T),
        "bk": np.ascontiguousarray(np.asarray(bk, np.float32).reshape(H_T, P).T),
        "bv": np.ascontiguousarray(np.asarray(bv, np.float32).reshape(H_T, P).T),
        "bo": np.ascontiguousarray(np.asarray(bo, np.float32).reshape(D_T, P).T),
    }
    maps = []
    for c in range(8):
        b, h = divmod(c, 2)
        m = dict(com)
        m["xT"] = np.ascontiguousarray(xT[b][:, h * NQ : (h + 1) * NQ])
        maps.append(m)
    return maps


def kernel(x, Wq, bq, Wk, bk, Wv, bv, Wo, bo, _trace=False, _precise=None):
    nc = _get_nc()
    maps = _in_maps(x, Wq, bq, Wk, bk, Wv, bv, Wo, bo)
    res = run_bass_kernel_spmd(nc, maps, list(range(8)), trace=_trace)
    out = np.empty((B, S, D), np.float32)
    for c in range(8):
        b, h = divmod(c, 2)
        out[b, h * NQ : (h + 1) * NQ, :] = res.results[c]["zT"].T
    if _trace:
        kernel.last_exec_time_ns = res.exec_time_ns
        kernel.last_profile = res
    return out


# revision 30
# speedup vs baseline: 1.1230x; 1.0983x over previous
"""Trainium2 Bass kernel for single-head attention (B=4, S=2048, D=H=1024).

Sharding: 8 cores = 4 batches x 2 sequence-halves. Each core projects
Q/K/V only for its OWN 1024-row half; the K^T and V halves are exchanged
with the pair partner via two pair-wise AllGather collectives (K first,
then V), each ~29us for 2 MiB and fully hidden behind the V/Q projection
matmuls. Scores/softmax/attnV/out then run on the core's own 1024 queries
against the full 2048-key sequence.

All matmuls single-pass fp16 (PSUM fp32 accumulation); rel l2 vs the fp32
reference is ~3e-3 (softmax logits ~N(0, 32^2), fp16 q/k rounding perturbs
logits by ~0.02 which the peaked softmax amplifies to ~2% absmax).

Per-core pipeline (v6):
  P0: x local half (2 MiB) + weights prefetched (ACT HWDGE queue for
      weights, SP for x; wv/wo/biases deferred past the first users).
  K:  KT_loc = Wk^T x -> staging -> kv_in_k -> AllGather -> KT full
      [h, 2048] SBUF tiles (both halves reloaded; position-independent
      so the SPMD program is identical across cores).
  V:  V_loc = x^T Wv -> staging -> kv_in_v -> AllGather -> V full
      [k, h] SBUF tiles. Runs while the K collective is in flight.
  Q:  Q = Wq^T x -> DRAM [H_T, QC, 128, 512] blocks. Runs while the V
      collective is in flight.
  B:  per 128-query tile: S = QT^T KT -> rowmax (negate) -> Exp(bias=-max,
      accum_out=den) -> En*recip -> 16 PE transposes (128x128, fp16 PSUM,
      packed 4 per bank) -> strided DVE copies into the per-chunk E^T
      supertile (no DRAM round trip; chunk-0 E^T complete mid-B).
  C:  per 512-query chunk: yT = V^T ET (+bv via softmax-sum=1),
      z^T = Wo^T yT (+bo) -> DRAM. Host transposes z^T back to [B,S,D].
"""

import sys

import numpy as np

for _p in ("/opt/trn_rl_repo",):
    if _p not in sys.path:
        sys.path.insert(0, _p)

import concourse.bass as bass
import concourse.masks as masks
import concourse.mybir as mybir
import concourse.tile as tile
from concourse.bass_utils import run_bass_kernel_spmd


def _install_profile_shims():
    """This image's `antenv` lacks `axon_hooks`, which run_bass_kernel_spmd
    imports for trace=True under axon; libaxon_pjrt.so has the NTFF symbols.
    Register a stand-in module wired to the ctypes hook, and neuter the
    artifact upload (zero-egress container)."""
    import types

    try:
        import antenv.axon_hooks  # noqa: F401
    except ImportError:
        hook = None
        try:
            import trn_agent_boot.trn_boot as _tb

            hook = _tb._ntff_profile_via_ctypes("/opt/axon/libaxon_pjrt.so")
        except Exception:
            hook = None
        import antenv

        m = types.ModuleType("antenv.axon_hooks")
        m.get_axon_ntff_profile_hook = lambda: hook
        m.set_axon_ntff_profile_hook = lambda h: None
        sys.modules["antenv.axon_hooks"] = m
        antenv.axon_hooks = m

    import concourse.bass_utils as _bu

    _bu.upload_artifacts = lambda tmpdir: tmpdir


_install_profile_shims()

B, S, D, H = 4, 2048, 1024, 1024
P = 128
NQ = 1024  # query rows per core == local key rows per core
D_T, H_T, S_T, Q_T = D // P, H // P, S // P, NQ // P
KC, QC, HC = S // 512, NQ // 512, H // 512
LC = NQ // 512  # local chunks

F32 = mybir.dt.float32
F16 = mybir.dt.float16
Ident = mybir.ActivationFunctionType.Identity
PAIRS = [[0, 1], [2, 3], [4, 5], [6, 7]]


def _split_multi_waits(nc, max_waits=1):
    """This container's walrus rejects >1 sync wait on NO_STRUCT opcodes
    (Drain/NoOp). Move extra waits onto dedicated single-wait NoOps inserted
    right before the offending instruction on the same engine."""
    for f in nc.m.functions:
        for bb in f.blocks:
            insts = bb.instructions
            i = 0
            while i < len(insts):
                ins = insts[i]
                si = ins.sync_info
                if si is not None and si.on_wait and len(si.on_wait) > max_waits:
                    waits = list(si.on_wait)
                    si.on_wait = waits[:max_waits]
                    ins.sync_info = si
                    for j, w in enumerate(waits[max_waits:]):
                        nop = mybir.InstNoOp(
                            name=f"{ins.name}-waitsplit-{j}",
                            engine=ins.engine,
                            bass_nofuse=True,
                            sync_info=mybir.SyncInfo(on_wait=[w], on_update=[]),
                        )
                        insts.insert(i, nop)
                        i += 1
                i += 1
            bb.instructions = insts


def _build(split_waits=True):
    nc = bass.Bass()

    def din(name, shape, dt=F16):
        return nc.declare_dram_parameter(name, shape, dt, isOutput=False)

    xT = din("xT", [D, NQ])  # this core's sequence half, [d, s_local]
    wq = din("wq", [D, H])
    wk = din("wk", [D, H])
    wv = din("wv", [D, H])
    wo = din("wo", [H, D])
    # biases pre-transposed on host to [128, n_tiles] so loads are contiguous
    bq, bk = din("bq", [P, H_T], F32), din("bk", [P, H_T], F32)
    bv, bo = din("bv", [P, H_T], F32), din("bo", [P, D_T], F32)
    zT = nc.declare_dram_parameter("zT", [D, NQ], F32, isOutput=True)

    with tile.TileContext(nc) as tc:
        with (
            tc.tile_pool(name="pers", bufs=1) as pers,
            tc.tile_pool(name="dram", bufs=1, space="DRAM") as dramp,
            tc.tile_pool(name="ps", bufs=6, space="PSUM") as psp,
            tc.tile_pool(name="pt", bufs=2, space="PSUM") as ptp,
        ):
            bias_q = pers.tile([P, H_T], F32, tag="bq", name="bq")
            bias_k = pers.tile([P, H_T], F32, tag="bk", name="bk")
            bias_v = pers.tile([P, H_T], F32, tag="bv", name="bv")
            bias_o = pers.tile([P, D_T], F32, tag="bo", name="bo")
            ident = pers.tile([P, P], F16, tag="ident", name="ident")
            masks.make_identity(nc, ident[:])

            # Q stays SBUF-resident: [h, q] tiles read directly as score
            # stationary operands in B.
            # Collective staging: local K^T/V halves out, both halves back.
            kv_in_k = dramp.tile([P, H_T * NQ], F16, tag="cink", name="cink")
            kv_in_v = dramp.tile([P, Q_T * H], F16, tag="cinv", name="cinv")
            kv_out_k = dramp.tile([2, P, H_T * NQ], F16, tag="coutk", name="coutk")
            kv_out_v = dramp.tile([2, P, Q_T * H], F16, tag="coutv", name="coutv")

            # Persistent SBUF: K^T, V, chunk-0 E^T supertile, weights.
            KT = [pers.tile([P, S], F16, tag=f"kt{t}", name=f"kt{t}") for t in range(H_T)]
            QS = [pers.tile([P, NQ], F16, tag=f"qs{t}", name=f"qs{t}") for t in range(H_T)]
            V = [pers.tile([P, H], F16, tag=f"v{s}", name=f"v{s}") for s in range(S_T)]
            ETa = pers.tile([P, S_T * 512], F16, tag="eta", name="eta")
            # wk per-d (streams into the first matmuls); wv/wo as one-DMA
            # supertiles (their consumers start late enough)
            wks = [pers.tile([P, H], F16, tag=f"wk{d}", name=f"wk{d}") for d in range(D_T)]
            wvs = pers.tile([P, D_T * H], F16, tag="wvs", name="wvs")
            wos = pers.tile([P, H_T * D], F16, tag="wos", name="wos")

            with tc.tile_pool(name="pqw", bufs=1) as pqw:
                wqs = pqw.tile([P, D_T * H], F16, tag="wqs", name="wqs")
                # biases first (tiny), then one supertile DMA per weight
                # matrix; x stream alone on the SP queue.
                for d in range(D_T):
                    nc.scalar.dma_start(out=wks[d][:], in_=wk[d * P : (d + 1) * P, :])
                nc.scalar.dma_start(out=bias_k[:], in_=bk[:, :])
                nc.scalar.dma_start(out=bias_q[:], in_=bq[:, :])
                nc.scalar.dma_start(out=bias_v[:], in_=bv[:, :])
                nc.scalar.dma_start(out=bias_o[:], in_=bo[:, :])
                nc.scalar.dma_start(
                    out=wqs[:].rearrange("p (d h) -> p d h", h=H),
                    in_=wq.rearrange("(d p) h -> p d h", p=P))
                nc.scalar.dma_start(
                    out=wvs[:].rearrange("p (d h) -> p d h", h=H),
                    in_=wv.rearrange("(d p) h -> p d h", p=P))
                nc.scalar.dma_start(
                    out=wos[:].rearrange("p (t d) -> p t d", d=D),
                    in_=wo.rearrange("(t p) d -> p t d", p=P))

                with tc.tile_pool(name="px", bufs=1) as px:
                    # x local half resident as 2 chunk supertiles [128, d*512]
                    # filled by per-d DMAs so the first matmuls can stream
                    xs = []
                    for c in range(LC):
                        cs = slice(c * 512, (c + 1) * 512)
                        t_ = px.tile([P, D_T * 512], F16, tag=f"x{c}",
                                     name=f"x{c}")
                        for d in range(D_T):
                            nc.sync.dma_start(
                                out=t_[:, d * 512 : (d + 1) * 512],
                                in_=xT[d * P : (d + 1) * P, cs])
                        xs.append(t_)

                    # ---- K: local K^T -> staging -> AllGather ------------
                    KTW = px.tile([P, H_T * NQ], F16, tag="stg", name="ktw")
                    for kc in range(LC):
                        for t in range(H_T):
                            ps = psp.tile([P, 512], F32, tag="ps", name="ps")
                            for d in range(D_T):
                                nc.tensor.matmul(
                                    ps[:],
                                    wks[d][:, t * P : (t + 1) * P],
                                    xs[kc][:, d * 512 : (d + 1) * 512],
                                    start=(d == 0), stop=(d == D_T - 1))
                            ws = slice(t * NQ + kc * 512, t * NQ + (kc + 1) * 512)
                            nc.scalar.activation(KTW[:, ws], ps[:], Ident,
                                                 bias=bias_k[:, t : t + 1])
                    nc.gpsimd.dma_start(out=kv_in_k[:, :], in_=KTW[:])
                    nc.gpsimd.collective_compute(
                        "AllGather", mybir.AluOpType.bypass,
                        replica_groups=PAIRS,
                        ins=[kv_in_k[:, :]],
                        outs=[kv_out_k[:, :, :]],
                    )
                    for t in range(H_T):
                        for p_ in range(2):
                            nc.sync.dma_start(
                                out=KT[t][:, p_ * NQ : (p_ + 1) * NQ],
                                in_=kv_out_k[p_, :, t * NQ : (t + 1) * NQ])

                    # ---- Q: local queries -> QS (SBUF resident) ----------
                    for qc in range(QC):
                        qcs = slice(qc * 512, (qc + 1) * 512)
                        for t in range(H_T):
                            ps = psp.tile([P, 512], F32, tag="ps", name="ps")
                            for d in range(D_T):
                                nc.tensor.matmul(
                                    ps[:],
                                    wqs[:, d * H + t * P : d * H + (t + 1) * P],
                                    xs[qc][:, d * 512 : (d + 1) * 512],
                                    start=(d == 0), stop=(d == D_T - 1))
                            nc.scalar.activation(QS[t][:, qcs], ps[:], Ident,
                                                 bias=bias_q[:, t : t + 1])

                    # ---- V: local V -> staging -> AllGather --------------
                    VTW = px.tile([P, Q_T * H], F16, tag="stg", name="vtw")
                    for kc in range(LC):
                        for si in range(4):
                            sl = kc * 4 + si
                            ksl = slice(si * P, (si + 1) * P)
                            for hc in range(HC):
                                hcs = slice(hc * 512, (hc + 1) * 512)
                                ps = psp.tile([P, 512], F32, tag="ps", name="ps")
                                for d in range(D_T):
                                    nc.tensor.matmul(
                                        ps[:],
                                        xs[kc][:, d * 512 + si * P : d * 512 + (si + 1) * P],
                                        wvs[:, d * H + hc * 512 : d * H + (hc + 1) * 512],
                                        start=(d == 0), stop=(d == D_T - 1))
                                # no +bv here: y = A(V + 1 bv^T) = AV + bv
                                # since softmax rows sum to 1; added in C.
                                ws = slice(sl * H + hc * 512,
                                           sl * H + (hc + 1) * 512)
                                nc.vector.tensor_copy(VTW[:, ws], ps[:])
                    nc.gpsimd.dma_start(out=kv_in_v[:, :], in_=VTW[:])
                    nc.gpsimd.collective_compute(
                        "AllGather", mybir.AluOpType.bypass,
                        replica_groups=PAIRS,
                        ins=[kv_in_v[:, :]],
                        outs=[kv_out_v[:, :, :]],
                    )
                    for s in range(S_T):
                        p_, sl = divmod(s, Q_T)
                        nc.sync.dma_start(
                            out=V[s][:],
                            in_=kv_out_v[p_, :, sl * H : (sl + 1) * H])


            # ---- B: scores + softmax + on-chip transpose -----------------
            with tc.tile_pool(name="pe2", bufs=1) as pe2:
                ETb = pe2.tile([P, S_T * 512], F16, tag="etb", name="etb")
                with tc.tile_pool(name="pb", bufs=1) as pb:
                    # Transposes run on the tensor engine, whose stream is
                    # in-order: emitted right after their own qt's scores
                    # they head-of-line-block the next qt's scores on the
                    # softmax-chain latency. Emit them one iteration behind,
                    # process qt3 last, and run C chunk 1 first so qt3's
                    # chain hides under chunk-1 attnV matmuls.
                    def transpose_emit(qt, Enn):
                        ET = ETa if qt < 4 else ETb
                        j = qt % 4
                        for a in range(4):
                            pst = ptp.tile([P, 512], F16, tag="pst", name="pst")
                            for i in range(4):
                                s = 4 * a + i
                                nc.tensor.transpose(
                                    pst[:, i * P : (i + 1) * P],
                                    Enn[:, s * P : (s + 1) * P],
                                    ident[:])
                            src = pst[:].rearrange("p (i c) -> p i c", c=P)
                            dst = ET[:].rearrange("p (s c) -> p s c", c=512)[
                                :, 4 * a : 4 * a + 4, j * P : (j + 1) * P]
                            nc.vector.tensor_copy(dst, src)

                    prev = None
                    for qt in (0, 1, 2, 4, 5, 6, 7, 3):
                        qs_ = slice(qt * P, (qt + 1) * P)
                        Ssb = pb.tile([P, S], F32, tag="Ssb", name="Ssb", bufs=2)
                        for kc in range(KC):
                            cs = slice(kc * 512, (kc + 1) * 512)
                            ps = psp.tile([P, 512], F32, tag="ps", name="ps")
                            for t in range(H_T):
                                nc.tensor.matmul(
                                    ps[:], QS[t][:, qs_], KT[t][:, cs],
                                    start=(t == 0), stop=(t == H_T - 1))
                            nc.vector.tensor_copy(Ssb[:, cs], ps[:])
                        if prev is not None:
                            transpose_emit(*prev)
                        nmx = pb.tile([P, 1], F32, tag="nmx", name="nmx", bufs=2)
                        nc.vector.reduce_max(nmx[:], Ssb[:],
                                             axis=mybir.AxisListType.X,
                                             negate=True)
                        En = pb.tile([P, S], F16, tag="En", name="En")
                        den = pb.tile([P, 1], F32, tag="den", name="den", bufs=2)
                        nc.scalar.activation(
                            En[:], Ssb[:], mybir.ActivationFunctionType.Exp,
                            bias=nmx[:], accum_out=den[:])
                        rec = pb.tile([P, 1], F32, tag="rec", name="rec", bufs=2)
                        nc.vector.reciprocal(rec[:], den[:])
                        # Enn lives in pe2: the last qt's transposes are
                        # emitted after this pool closes (inside C).
                        Enn = pe2.tile([P, S], F16, tag="Enn", name="Enn", bufs=2)
                        nc.scalar.mul(Enn[:], En[:], rec[:])
                        prev = (qt, Enn)

                # ---- C: per q-chunk: yT = V^T ET, z = Wo^T yT ------------
                with tc.tile_pool(name="pc", bufs=1) as pc:
                    for ci, qc in enumerate((1, 0)):
                        if ci == 1:
                            transpose_emit(*prev)  # qt3, hidden under chunk 1
                        cs = slice(qc * 512, (qc + 1) * 512)
                        ET = ETa if qc == 0 else ETb
                        ycs = []
                        for t in range(H_T):
                            hs = slice(t * P, (t + 1) * P)
                            ps = psp.tile([P, 512], F32, tag="ps", name="ps")
                            for s in range(S_T):
                                nc.tensor.matmul(
                                    ps[:], V[s][:, hs],
                                    ET[:, s * 512 : (s + 1) * 512],
                                    start=(s == 0), stop=(s == S_T - 1))
                            yc = pc.tile([P, 512], F16, tag=f"yc{t}",
                                         name=f"yc{t}", bufs=2)
                            nc.scalar.activation(yc[:], ps[:], Ident,
                                                 bias=bias_v[:, t : t + 1])
                            ycs.append(yc)
                        for td in range(D_T):
                            ds_ = slice(td * P, (td + 1) * P)
                            ps = psp.tile([P, 512], F32, tag="ps", name="ps")
                            for t in range(H_T):
                                nc.tensor.matmul(
                                    ps[:],
                                    wos[:, t * D + td * P : t * D + (td + 1) * P],
                                    ycs[t][:],
                                    start=(t == 0), stop=(t == H_T - 1))
                            zsb = pc.tile([P, 512], F32, tag="zsb", name="zsb",
                                          bufs=2)
                            nc.scalar.activation(zsb[:], ps[:], Ident,
                                                 bias=bias_o[:, td : td + 1])
                            nc.sync.dma_start(out=zT[ds_, cs], in_=zsb[:])

    if split_waits:
        _split_multi_waits(nc)
    return nc


_NC = {}


def _get_nc():
    if "v6" not in _NC:
        _NC["v6"] = _build()
    return _NC["v6"]


def _in_maps(x, Wq, bq, Wk, bk, Wv, bv, Wo, bo):
    x = np.asarray(x, np.float32)
    xT = np.transpose(x, (0, 2, 1)).astype(np.float16)  # [B, D, S]
    com = {
        "wq": np.asarray(Wq, np.float16),
        "wk": np.asarray(Wk, np.float16),
        "wv": np.asarray(Wv, np.float16),
        "wo": np.asarray(Wo, np.float16),
        "bq": np.ascontiguousarray(np.asarray(bq, np.float32).reshape(H_T, P).T),
        "bk": np.ascontiguousarray(np.asarray(bk, np.float32).reshape(H_T, P).T),
        "bv": np.ascontiguousarray(np.asarray(bv, np.float32).reshape(H_T, P).T),
        "bo": np.ascontiguousarray(np.asarray(bo, np.float32).reshape(D_T, P).T),
    }
    maps = []
    for c in range(8):
        b, h = divmod(c, 2)
        m = dict(com)
        m["xT"] = np.ascontiguousarray(xT[b][:, h * NQ : (h + 1) * NQ])
        maps.append(m)
    return maps


def kernel(x, Wq, bq, Wk, bk, Wv, bv, Wo, bo, _trace=False, _precise=None):
    nc = _get_nc()
    maps = _in_maps(x, Wq, bq, Wk, bk, Wv, bv, Wo, bo)
    res = run_bass_kernel_spmd(nc, maps, list(range(8)), trace=_trace)
    out = np.empty((B, S, D), np.float32)
    for c in range(8):
        b, h = divmod(c, 2)
        out[b, h * NQ : (h + 1) * NQ, :] = res.results[c]["zT"].T
    if _trace:
        kernel.last_exec_time_ns = res.exec_time_ns
        kernel.last_profile = res
    return out
